# revision 41
# baseline (speedup 1.0000x reference)
"""NonLocalBlock (B=8, C=256, HW=64x64) Trainium2 kernel.

Data-parallel over batch: one sample per NeuronCore (8 cores).
Per core (everything on-chip; the [N,N]=67MB attention matrix never
touches HBM):

  x [C=256, N=4096] fp16 --DMA--> SBUF (serves logit path, g path, residual)
  theta = w_theta @ x + b_theta    [O=128, N]  fp16   (PE fp16)
  phi   = w_phi   @ x + b_phi      [O=128, N]  fp16
  gt    = (w_g @ x)^T              [N, O] bf16 chunks + ones column
  per 512-wide n-tile (iteration it handles tile nt=it plus leftovers of
  tile pv=it-1):
    S^T chunks [m=128, n=512] = phi_chunk^T . theta_tile  (PE fp16 -> PSUM)
    P^T = exp(S^T): 3-chunk groups alternate between ScalarE (exact EXP
        ACTIVATE, PSUM->SBUF bf16) and DVE (Schraudolph fast-exp: one
        tensor_scalar mult-add producing int16 bits that ARE the bf16
        exp approximation, ~3% rel err - calibrated against the 2e-2
        gate). Splitting exp across two engines removes ScalarE as the
        pipeline pacer (exp is 109us of ScalarE work if unsplit).
    y[s] [128, 129] += P^T_chunk^T . [gt_chunk | ones]    (PE bf16)
        col 128 accumulates the softmax row-sum for free.
        Slices s=0,1 of tile nt run in iteration it lagging the S^T
        pipeline by TWO groups (so a 1.4-1.7us exp has ~2 pipeline
        positions of slack, not 1); slices s=2,3 of tile pv are
        front-loaded into positions 0..5 (all their P^T is ready), so
        the y-accumulator banks recycle half an iteration before their
        next writer - no PE stall + p-state ramp at tile boundaries.
    ynorm = y[:, :128] * (1/y[:,128])    (DVE)
    yt[o, n-sub] = DMA-xbar-transpose(ynorm) on the sync queue; tile 7
        instead transposes on the then-idle PE (is_transpose matmul vs
        an identity, staged through the spare tail of the y PSUM bank)
        - the 4 serialized DMA transposes were ~5us of pure tail latency
    z = w_out^T . yt at position 10 of iteration pv+1; bn on ScalarE
        (ACTIVATE Identity with per-partition scale+bias APs, deferred
        past exp(g10) in queue order); residual add on GpSimd (DVE for
        tile 7); out DMA split across sync/gpsimd queues. Tile 7's
        output projection is split into two 256-col halves so the first
        half runs right after the final mm2 burst.
       (b_g is folded into bn_shift' on the host: w_out @ b_g is constant)

Prologue: x loads stream as 16 x 128KB per-block DMAs split across the
sync and gpsimd queues (single-queue load serialized 11us before the
first matmul), consts on the scalar HWDGE queue ordered by first use;
theta/phi/gt projections pipeline per 512-col block, with each block's
DMA triggers emitted interleaved one block ahead of the consuming
matmuls (emitting all loads upfront stalled the sync HWDGE ring at 4
in-flight and put the first matmul behind a coarse ~4.5us semaphore).
Iteration 0's first exp groups are split across both engines to halve
exp latency during pipeline fill (no pv work exists yet to hide it).

PSUM (8 banks): S^T groups of 3 chunks [128,1536] double-buffered (6 banks)
+ a 2-slot ring where two [128,129] y-accumulators pack into one bank (the
second starts with start=False onto the bank cleared by the first's
start=True) and the output-projection tiles reuse the same slots.
"""
import os
import sys

sys.path.insert(0, "/opt/trn_rl_repo")

import numpy as np
import ml_dtypes

import concourse.bass as bass
import concourse.bacc as bacc
import concourse.mybir as mybir
import concourse.tile as tile
from concourse.tile import add_dep_helper
from concourse.bass_utils import run_bass_kernel_spmd

F32 = mybir.dt.float32
F16 = mybir.dt.float16
BF16 = mybir.dt.bfloat16
I16 = mybir.dt.int16
ADD = mybir.AluOpType.add
MULT = mybir.AluOpType.mult
EXP = mybir.ActivationFunctionType.Exp
IDENT = mybir.ActivationFunctionType.Identity

B, C, O, N = 8, 256, 128, 4096
NT = 512
N_TILES = N // NT            # 8
M_CHUNKS = N // 128          # 32
# S^T groups: chunks per exp ACTIVATE (3 banks => double-buffered in 6)
GROUPS = [(c, min(c + 3, M_CHUNKS)) for c in range(0, M_CHUNKS, 3)]  # 11 groups
# groups whose exp runs as a DVE fast-exp instead of a ScalarE ACTIVATE
DVE_GROUPS = {1, 3, 5, 7, 10}
# Schraudolph fast-exp into bf16 bits: i16 = trunc(S*A + Bc); bits are bf16.
# c=0.0436 calibrated for truncation semantics; validated rel err 0.007.
FEXP_A = float(128.0 / np.log(2.0))
FEXP_B = float(128.0 * (127.0 - 0.0436))
BN_EPS = 1e-5


def build_nc():
    nc = bacc.Bacc()

    xin = nc.dram_tensor("xin", [C, N], F16, kind="ExternalInput")
    wth = nc.dram_tensor("wth", [C, O], F16, kind="ExternalInput")     # w_theta.T
    wph = nc.dram_tensor("wph", [C, O], F16, kind="ExternalInput")     # w_phi.T
    wg = nc.dram_tensor("wg", [C, O], F16, kind="ExternalInput")       # w_g.T
    wout = nc.dram_tensor("wout", [O, C], BF16, kind="ExternalInput")  # w_out.T
    bth = nc.dram_tensor("bth", [O, 1], F32, kind="ExternalInput")
    bph = nc.dram_tensor("bph", [O, 1], F32, kind="ExternalInput")
    identm = nc.dram_tensor("identm", [128, 128], BF16, kind="ExternalInput")
    bnscale = nc.dram_tensor("bnscale", [128, 2], F32, kind="ExternalInput")
    bnshift = nc.dram_tensor("bnshift", [128, 2], F32, kind="ExternalInput")
    out = nc.dram_tensor("out", [C, N], F32, kind="ExternalOutput")

    with tile.TileContext(nc) as tc:
        with tc.tile_pool(name="const", bufs=1) as const, \
             tc.tile_pool(name="xpool", bufs=1) as xpool, \
             tc.tile_pool(name="proj", bufs=1) as proj, \
             tc.tile_pool(name="yt_pool", bufs=3) as yt_pool, \
             tc.tile_pool(name="small", bufs=6) as small, \
             tc.tile_pool(name="ostage", bufs=6) as ostage:

            # ---- constants on the scalar HWDGE queue (each DMA has ~600ns
            # fixed queue occupancy; keeping them off sync/gpsimd lets the x
            # blocks start streaming immediately) ----
            wth_sb = const.tile([128, 2 * O], F16)
            wph_sb = const.tile([128, 2 * O], F16)
            wg_sb = const.tile([128, 2 * O], F16)
            bth_sb = const.tile([O, 1], F32)
            bph_sb = const.tile([O, 1], F32)
            for k in range(2):
                csl = slice(k * 128, (k + 1) * 128)
                osl = slice(k * O, (k + 1) * O)
                nc.scalar.dma_start(wth_sb[:, osl], wth[csl, :])
                nc.scalar.dma_start(wg_sb[:, osl], wg[csl, :])
                nc.scalar.dma_start(wph_sb[:, osl], wph[csl, :])
            nc.scalar.dma_start(bth_sb[:], bth[:])
            nc.scalar.dma_start(bph_sb[:], bph[:])
            wout_sb = const.tile([O, C], BF16)
            nc.scalar.dma_start(wout_sb[:], wout[:])
            bnscale_sb = const.tile([128, 2], F32)
            bnshift_sb = const.tile([128, 2], F32)
            nc.scalar.dma_start(bnscale_sb[:], bnscale[:])
            nc.scalar.dma_start(bnshift_sb[:], bnshift[:])
            ident_sb = const.tile([128, 128], BF16)
            nc.scalar.dma_start(ident_sb[:], identm[:])

            # ---- x load: one tile per (half, 512-col block) so consumers
            # depend on exactly the DMA they need, halves on 2 queues ----
            x_blk = [[xpool.tile([128, NT], F16, name=f"x{k}_{b}")
                      for b in range(N_TILES)] for k in range(2)]

            theta_h = proj.tile([O, N], F16)
            phi_h = proj.tile([O, N], F16)
            gt_sb = proj.tile([128, M_CHUNKS * (O + 1)], BF16)
            # preload the EXP activation table before the first real exp
            exp_warm = small.tile([O, 1], F32)
            nc.scalar.activation(exp_warm[:], bth_sb[:], EXP)

            # ---- projections, pipelined per 512-col block; DMA triggers are
            # emitted interleaved with the consuming matmuls (one block ahead)
            # so completion semaphores bind per block - emitting all 16 loads
            # upfront made the first matmul wait ~4.5us on a coarse semaphore
            # and stalled the sync HWDGE ring at 4 in-flight ----
            with tc.tile_pool(name="thph_ps", bufs=2, space="PSUM") as thph_ps, \
                 tc.tile_pool(name="gt_ps", bufs=2, space="PSUM") as gt_ps:
                for b in range(-1, N_TILES):
                    if b + 1 < N_TILES:
                        nbsl = slice((b + 1) * NT, (b + 2) * NT)
                        nc.sync.dma_start(x_blk[0][b + 1][:], xin[0:128, nbsl])
                        nc.gpsimd.dma_start(x_blk[1][b + 1][:], xin[128:256, nbsl])
                    if b < 0:
                        continue
                    bsl = slice(b * NT, (b + 1) * NT)
                    pp = thph_ps.tile([128, 2 * NT], F32)
                    pth, pph = pp[:, 0:NT], pp[:, NT:2 * NT]
                    for k in range(2):
                        nc.tensor.matmul(
                            pth, wth_sb[:, k * O:(k + 1) * O],
                            x_blk[k][b][:], start=(k == 0), stop=(k == 1),
                        )
                    for k in range(2):
                        nc.tensor.matmul(
                            pph, wph_sb[:, k * O:(k + 1) * O],
                            x_blk[k][b][:], start=(k == 0), stop=(k == 1),
                        )
                    if b % 2 == 0:
                        nc.scalar.activation(theta_h[:, bsl], pth, IDENT, bias=bth_sb[:])
                        nc.vector.tensor_scalar_add(phi_h[:, bsl], pph, bph_sb[:])
                    else:
                        nc.vector.tensor_scalar_add(theta_h[:, bsl], pth, bth_sb[:])
                        nc.scalar.activation(phi_h[:, bsl], pph, IDENT, bias=bph_sb[:])
                    for c in range(4 * b, 4 * b + 4):
                        gp = gt_ps.tile([128, O], F32)
                        q = (c % 4) * 128
                        for k in range(2):
                            nc.tensor.matmul(
                                gp[:],
                                x_blk[k][b][:, q:q + 128],
                                wg_sb[:, k * O:(k + 1) * O],
                                start=(k == 0), stop=(k == 1),
                            )
                        base = c * (O + 1)
                        if c % 2 == 0:
                            nc.vector.tensor_copy(gt_sb[:, base:base + O], gp[:])
                        else:
                            nc.scalar.copy(gt_sb[:, base:base + O], gp[:])
                        # softmax-denominator ones column (gpsimd, off the
                        # critical path, behind this block's dma triggers)
                        nc.gpsimd.memset(
                            gt_sb[:, base + O:base + O + 1], 1.0)

            # ---- software-pipelined attention ----
            # Iteration it: S^T+exp+y01(s0,s1) mm2 for tile nt=it, with tile
            # pv=it-1's y23(s2,s3) mm2 FRONT-LOADED into positions 0..5 (its
            # P^T tiles are all ready), its norms+transposes at position 6 and
            # its output projection at position 9 of the same iteration. This
            # keeps the y-accumulator PSUM banks a full half-iteration ahead
            # of their next writer (the old layout stalled the PE ~1.4us at
            # every tile boundary waiting for end-of-tile norms, then paid
            # ~3us of half-clock p-state ramp).
            with tc.tile_pool(name="pt_pool", bufs=22) as pt_pool, \
                 tc.tile_pool(name="st_ps", bufs=2, space="PSUM") as st_ps, \
                 tc.tile_pool(name="ya", bufs=2, space="PSUM") as ya:

                def mm2(y, pt, s, c0, c1, pt_c0):
                    # y accumulator slice gets chunks [c0, c1) of P^T tile pt.
                    # Two accumulators share one PSUM bank: only the first
                    # (s even) opens the group (start=True clears the whole
                    # bank); the second writes start=False onto cleared bits.
                    first = None
                    for c in range(c0, c1):
                        i = nc.tensor.matmul(
                            y[:],
                            pt[:, (c - pt_c0) * NT + s * 128:(c - pt_c0) * NT + (s + 1) * 128],
                            gt_sb[:, c * (O + 1):(c + 1) * (O + 1)],
                            start=(c == 0 and s % 2 == 0),
                            stop=(c == M_CHUNKS - 1),
                            skip_group_check=True,
                        )
                        if first is None:
                            first = i
                    return first

                def norm_transpose(y, yt_tile, col, q):
                    recip = small.tile([128, 1], F32)
                    nc.vector.reciprocal(recip[:], y[:, O:O + 1])
                    ynorm = small.tile([128, O], BF16)
                    nc.vector.tensor_scalar_mul(ynorm[:], y[:, 0:O], recip[:])
                    q.dma_start_transpose(yt_tile[:, col:col + 128], ynorm[:])

                def norm_pair_pe(ytile, yt_tile, colbase, copy_eng):
                    # last-tile path: transpose on the (idle) PE instead of the
                    # DMA xbar, staging through the spare tail of the y PSUM
                    # bank; the serialized DMA transposes were ~5us of pure
                    # tail latency. start=True zeroes the whole bank, so the
                    # first transpose must wait for BOTH slices' norm reads.
                    yn = []
                    for lo in (0, 130):
                        recip = small.tile([128, 1], F32)
                        nc.vector.reciprocal(recip[:], ytile[:, lo + O:lo + O + 1])
                        ynorm = small.tile([128, O], BF16)
                        mi = nc.vector.tensor_scalar_mul(
                            ynorm[:], ytile[:, lo:lo + O], recip[:])
                        yn.append((ynorm, mi))
                    dstb = ytile[:, 260:392].bitcast(BF16)
                    prev = None
                    for j, (ynorm, mi) in enumerate(yn):
                        dst = dstb[:, j * 132:j * 132 + 128]
                        im = nc.tensor.matmul(
                            dst, ynorm[:], ident_sb[:], is_transpose=True,
                            start=(j == 0), stop=True, skip_group_check=True,
                        )
                        if j == 0:
                            add_dep_helper(im.ins, yn[1][1].ins, sync=True,
                                           reason="bank clear only after all norm reads")
                        else:
                            add_dep_helper(im.ins, prev.ins, sync=False,
                                           reason="bank-pack: clear before second write")
                        prev = im
                        if copy_eng is nc.scalar:
                            nc.scalar.copy(
                                yt_tile[:, colbase + j * 128:colbase + (j + 1) * 128], dst)
                        else:
                            copy_eng.tensor_copy(
                                yt_tile[:, colbase + j * 128:colbase + (j + 1) * 128], dst)

                def bn_res_dma(src_ap, ct, t, off, width):
                    # bn on ScalarE: Identity(z*scale + bias) with per-partition
                    # APs; residual add on GpSimd (DVE for the last tile, whose
                    # tail has DVE idle); out DMA split across queues.
                    obn = ostage.tile([128, width], F32)
                    nc.scalar.activation(
                        obn[:], src_ap, IDENT,
                        bias=bnshift_sb[:, ct:ct + 1], scale=bnscale_sb[:, ct:ct + 1],
                    )
                    ores = ostage.tile([128, width], F32, name="ores")
                    eng = nc.vector if t == N_TILES - 1 else nc.gpsimd
                    eng.tensor_tensor(
                        ores[:], obn[:], x_blk[ct][t][:, off:off + width], op=ADD
                    )
                    # last tile: keep everything on sync so the gpsimd DMA
                    # ring has nothing late to drain in the epilogue
                    q = nc.sync if (ct == 0 or t == N_TILES - 1) else nc.gpsimd
                    q.dma_start(
                        out[ct * 128:(ct + 1) * 128, t * NT + off:t * NT + off + width],
                        ores[:])

                def out_proj_mm_full(t, yt_tile):
                    o01 = st_ps.tile([128, 1024], F32, name="o01", tag="st")
                    for ct in range(2):
                        nc.tensor.matmul(
                            o01[:, ct * NT:(ct + 1) * NT],
                            wout_sb[:, ct * 128:(ct + 1) * 128],
                            yt_tile[:],
                            start=True, stop=True,
                            skip_group_check=True,
                        )
                    return [(o01[:, ct * NT:(ct + 1) * NT], ct, t, 0, NT)
                            for ct in range(2)]

                def out_proj_mm_half(t, h, yt_tile):
                    oh = st_ps.tile([128, 512], F32, name="oh", tag="st")
                    csl = slice(h * 256, (h + 1) * 256)
                    for ct in range(2):
                        nc.tensor.matmul(
                            oh[:, ct * 256:(ct + 1) * 256],
                            wout_sb[:, ct * 128:(ct + 1) * 128],
                            yt_tile[:, csl],
                            start=True, stop=True,
                            skip_group_check=True,
                        )
                    return [(oh[:, ct * 256:(ct + 1) * 256], ct, t, h * 256, 256)
                            for ct in range(2)]

                def y23_groups(pos):
                    # front-load the 11 groups of the previous tile's y23 mm2
                    # into positions 0..5 (their P^T tiles are all ready)
                    if pos < 5:
                        return [2 * pos, 2 * pos + 1]
                    if pos == 5:
                        return [10]
                    return []

                pts_prev = None
                yts = {}
                n_grps = len(GROUPS)
                last = N_TILES - 1
                for it in range(N_TILES + 1):
                    nt = it if it < N_TILES else None
                    pv = it - 1 if it >= 1 else None
                    if nt is not None:
                        ntsl = slice(nt * NT, (nt + 1) * NT)
                        y01 = ya.tile([128, 392], F32, name="y01", tag="ya")
                        yts[nt] = yt_pool.tile([O, NT], BF16, name=f"yt{nt}")
                    if pv is not None:
                        y23 = ya.tile([128, 392], F32, name="y23", tag="ya")
                    pts_cur = []
                    bns = []
                    for g in range(n_grps):
                        if pv is not None and g == 0:
                            # position 0: previous tile's y23 mm2 first (deps
                            # long met) so the PE has work while the previous
                            # out-projection's bn reads free the S^T-ring slot
                            for gg in y23_groups(0):
                                c0, c1 = GROUPS[gg]
                                ia = mm2(y23[:, 0:O + 1], pts_prev[gg], 2, c0, c1, c0)
                                ib = mm2(y23[:, 130:259], pts_prev[gg], 3, c0, c1, c0)
                                if c0 == 0:
                                    add_dep_helper(ib.ins, ia.ins, sync=False,
                                                   reason="bank-pack: clear before first write")
                        if nt is not None:
                            c0, c1 = GROUPS[g]
                            w = (c1 - c0) * NT
                            st = st_ps.tile([128, 3 * NT], F32, name="st", tag="st")
                            for c in range(c0, c1):
                                nc.tensor.matmul(
                                    st[:, (c - c0) * NT:(c - c0 + 1) * NT],
                                    phi_h[:, c * 128:(c + 1) * 128],
                                    theta_h[:, ntsl],
                                    start=True, stop=True,
                                )
                            pt = pt_pool.tile([128, 3 * NT], BF16)
                            if it == 0 and g < 4:
                                # pipeline fill: no pv work exists to hide a
                                # whole-group exp latency, so halve it by
                                # splitting the group across both engines
                                nc.scalar.activation(pt[:, :2 * NT], st[:, :2 * NT], EXP)
                                nc.vector.tensor_scalar(
                                    pt[:, 2 * NT:w].bitcast(I16), st[:, 2 * NT:w],
                                    FEXP_A, FEXP_B, op0=MULT, op1=ADD,
                                )
                            elif g in DVE_GROUPS:
                                nc.vector.tensor_scalar(
                                    pt[:, :w].bitcast(I16), st[:, :w],
                                    FEXP_A, FEXP_B, op0=MULT, op1=ADD,
                                )
                            else:
                                nc.scalar.activation(pt[:, :w], st[:, :w], EXP)
                            pts_cur.append(pt)
                            if g > 1:
                                # y01 mm2 lags TWO groups behind S^T: exp(g)
                                # then has ~2 pipeline positions before its
                                # consumer instead of 1 (a 3-chunk exp takes
                                # 1.4-1.7us but a back-half position is ~1us)
                                pc0, pc1 = GROUPS[g - 2]
                                ia = mm2(y01[:, 0:O + 1], pts_cur[g - 2], 0, pc0, pc1, pc0)
                                ib = mm2(y01[:, 130:259], pts_cur[g - 2], 1, pc0, pc1, pc0)
                                if pc0 == 0:
                                    add_dep_helper(ib.ins, ia.ins, sync=False,
                                                   reason="bank-pack: clear before first write")
                        if pv is not None and g > 0:
                            for gg in y23_groups(g):
                                c0, c1 = GROUPS[gg]
                                ia = mm2(y23[:, 0:O + 1], pts_prev[gg], 2, c0, c1, c0)
                                ib = mm2(y23[:, 130:259], pts_prev[gg], 3, c0, c1, c0)
                                if c0 == 0:
                                    add_dep_helper(ib.ins, ia.ins, sync=False,
                                                   reason="bank-pack: clear before first write")
                        if pv is not None and g == 6:
                            if pv == last:
                                # tail: first half of tile 7's output projection
                                # (yt cols 0:256 transposed during the previous
                                # iteration) straight after the final mm2 burst
                                bns += out_proj_mm_half(pv, 0, yts[pv])
                                norm_pair_pe(y23, yts[pv], 256, nc.scalar)
                            else:
                                norm_transpose(y23[:, 0:O + 1], yts[pv], 256, nc.sync)
                                norm_transpose(y23[:, 130:259], yts[pv], 384, nc.sync)
                        if pv is not None and g == 10:
                            if pv < last:
                                bns += out_proj_mm_full(pv, yts[pv])
                            else:
                                bns += out_proj_mm_half(pv, 1, yts[pv])
                    if nt is not None:
                        for gg in (n_grps - 2, n_grps - 1):
                            c0, c1 = GROUPS[gg]
                            mm2(y01[:, 0:O + 1], pts_cur[gg], 0, c0, c1, c0)
                            mm2(y01[:, 130:259], pts_cur[gg], 1, c0, c1, c0)
                    for args in bns:
                        bn_res_dma(*args)
                    if nt is not None:
                        if nt == last:
                            # copies on DVE: scalar is still draining exp/bn
                            # residue here, and these copies gate h0
                            norm_pair_pe(y01, yts[nt], 0, nc.vector)
                        else:
                            norm_transpose(y01[:, 0:O + 1], yts[nt], 0, nc.sync)
                            norm_transpose(y01[:, 130:259], yts[nt], 128, nc.sync)
                    pts_prev = pts_cur

    nc.finalize()
    return nc


_NC_CACHE = None


def _get_nc():
    global _NC_CACHE
    if _NC_CACHE is None:
        _NC_CACHE = build_nc()
    return _NC_CACHE


def _prepare_in_maps(inputs):
    x = np.ascontiguousarray(np.asarray(inputs["x"], dtype=np.float32)).reshape(B, C, N)
    xh = x.astype(np.float16)
    wth = np.ascontiguousarray(np.asarray(inputs["w_theta"], np.float32).T).astype(np.float16)
    wph = np.ascontiguousarray(np.asarray(inputs["w_phi"], np.float32).T).astype(np.float16)
    wg = np.ascontiguousarray(np.asarray(inputs["w_g"], np.float32).T).astype(np.float16)
    w_out = np.asarray(inputs["w_out"], np.float32)
    wout = np.ascontiguousarray(w_out.T).astype(ml_dtypes.bfloat16)
    bth = np.asarray(inputs["b_theta"], np.float32).reshape(O, 1)
    bph = np.asarray(inputs["b_phi"], np.float32).reshape(O, 1)
    inv = np.asarray(inputs["bn_gamma"], np.float32) / np.sqrt(
        np.asarray(inputs["bn_var"], np.float32) + BN_EPS)
    shift = (np.asarray(inputs["b_out"], np.float32) * inv
             + np.asarray(inputs["bn_beta"], np.float32)
             - np.asarray(inputs["bn_mean"], np.float32) * inv)
    # fold the g-branch bias through the output projection: softmax rows sum
    # to 1, so attn @ (g + b_g) = attn @ g + b_g, and w_out @ b_g is constant
    wob = wout.astype(np.float32) .T @ np.asarray(inputs["b_g"], np.float32)
    shift = shift + inv * wob
    bnscale = np.ascontiguousarray(inv.reshape(2, 128).T)
    bnshift = np.ascontiguousarray(shift.reshape(2, 128).T)

    shared = dict(wth=wth, wph=wph, wg=wg, wout=wout, bth=bth, bph=bph,
                  bnscale=bnscale, bnshift=bnshift,
                  identm=np.eye(128, dtype=ml_dtypes.bfloat16))
    return [dict(shared, xin=np.ascontiguousarray(xh[b])) for b in range(B)]


def _install_ntff_shim():
    """This image's antenv lacks axon_hooks; provide it from trn_boot's
    ctypes implementation so trace=True can capture NTFF profiles."""
    import types
    try:
        import antenv.axon_hooks  # noqa: F401
        return
    except ImportError:
        pass
    if "/root/.axon_site" not in sys.path:
        sys.path.insert(0, "/root/.axon_site")
    from trn_agent_boot.trn_boot import _ntff_profile_via_ctypes
    hook = _ntff_profile_via_ctypes("/opt/axon/libaxon_pjrt.so")
    m = types.ModuleType("antenv.axon_hooks")
    m.get_axon_ntff_profile_hook = lambda: hook
    m.set_axon_ntff_profile_hook = lambda h: None
    sys.modules["antenv.axon_hooks"] = m


def run(inputs, trace=False):
    if trace:
        _install_ntff_shim()
    nc = _get_nc()
    in_maps = _prepare_in_maps(inputs)
    res = run_bass_kernel_spmd(nc, in_maps, list(range(B)), trace=trace)
    outs = np.stack([res.results[b]["out"] for b in range(B)])
    return outs.reshape(B, C, 64, 64), res


def kernel(**inputs) -> np.ndarray:
    out, _ = run(inputs)
    return out


if __name__ == "__main__":
    # quick CoreSim check of one core
    from concourse import bass_interp
    rng = np.random.default_rng(0)
    fake = {
        "x": rng.standard_normal((B, C, 64, 64)).astype(np.float32),
        "w_theta": (rng.standard_normal((O, C)) * 0.05).astype(np.float32),
        "b_theta": (rng.standard_normal(O) * 0.05).astype(np.float32),
        "w_phi": (rng.standard_normal((O, C)) * 0.05).astype(np.float32),
        "b_phi": (rng.standard_normal(O) * 0.05).astype(np.float32),
        "w_g": (rng.standard_normal((O, C)) * 0.05).astype(np.float32),
        "b_g": (rng.standard_normal(O) * 0.05).astype(np.float32),
        "w_out": (rng.standard_normal((C, O)) * 0.05).astype(np.float32),
        "b_out": (rng.standard_normal(C) * 0.05).astype(np.float32),
        "bn_gamma": rng.standard_normal(C).astype(np.float32),
        "bn_beta": rng.standard_normal(C).astype(np.float32),
        "bn_mean": rng.standard_normal(C).astype(np.float32),
        "bn_var": rng.uniform(0.5, 1.5, C).astype(np.float32),
    }
    nc = _get_nc()
    in_maps = _prepare_in_maps(fake)
    sim = bass_interp.CoreSim(nc)
    for k, v in in_maps[0].items():
        sim.tensor(k)[:] = v
    sim.simulate()
    got = np.asarray(sim.tensor("out"))

    x0 = fake["x"][0].reshape(C, N)
    th = fake["w_theta"] @ x0 + fake["b_theta"][:, None]
    ph = fake["w_phi"] @ x0 + fake["b_phi"][:, None]
    gg = fake["w_g"] @ x0 + fake["b_g"][:, None]
    s = th.T @ ph
    p = np.exp(s - s.max(1, keepdims=True))
    a = p / p.sum(1, keepdims=True)
    yy = a @ gg.T
    wy = fake["w_out"] @ yy.T + fake["b_out"][:, None]
    inv = fake["bn_gamma"] / np.sqrt(fake["bn_var"] + BN_EPS)
    bn = wy * inv[:, None] + (fake["bn_beta"] - fake["bn_mean"] * inv)[:, None]
    want = x0 + bn
    err = np.abs(got - want).max()
    print("CoreSim absmax err:", err, "rel:", err / np.abs(want).max())


# revision 43
# speedup vs baseline: 1.0048x; 1.0048x over previous
"""NonLocalBlock (B=8, C=256, HW=64x64) Trainium2 kernel.

Data-parallel over batch: one sample per NeuronCore (8 cores).
Per core (everything on-chip; the [N,N]=67MB attention matrix never
touches HBM):

  x [C=256, N=4096] fp16 --DMA--> SBUF (serves logit path, g path, residual)
  theta = w_theta @ x + b_theta    [O=128, N]  fp16   (PE fp16)
  phi   = w_phi   @ x + b_phi      [O=128, N]  fp16
  gt    = (w_g @ x)^T              [N, O] bf16 chunks + ones column
  per 512-wide n-tile (iteration it handles tile nt=it plus leftovers of
  tile pv=it-1):
    S^T chunks [m=128, n=512] = phi_chunk^T . theta_tile  (PE fp16 -> PSUM)
    P^T = exp(S^T): 3-chunk groups alternate between ScalarE (exact EXP
        ACTIVATE, PSUM->SBUF bf16) and DVE (Schraudolph fast-exp: one
        tensor_scalar mult-add producing int16 bits that ARE the bf16
        exp approximation, ~3% rel err - calibrated against the 2e-2
        gate). Splitting exp across two engines removes ScalarE as the
        pipeline pacer (exp is 109us of ScalarE work if unsplit).
    y[s] [128, 129] += P^T_chunk^T . [gt_chunk | ones]    (PE bf16)
        col 128 accumulates the softmax row-sum for free.
        Slices s=0,1 of tile nt run in iteration it lagging the S^T
        pipeline by TWO groups (so a 1.4-1.7us exp has ~2 pipeline
        positions of slack, not 1); slices s=2,3 of tile pv are
        front-loaded into positions 0..5 (all their P^T is ready), so
        the y-accumulator banks recycle half an iteration before their
        next writer - no PE stall + p-state ramp at tile boundaries.
    ynorm = y[:, :128] * (1/y[:,128])    (DVE)
    yt[o, n-sub] = DMA-xbar-transpose(ynorm) on the sync queue; tile 7
        instead transposes on the then-idle PE (is_transpose matmul vs
        an identity, staged through the spare tail of the y PSUM bank)
        - the 4 serialized DMA transposes were ~5us of pure tail latency
    z = w_out^T . yt at position 10 of iteration pv+1; bn on ScalarE
        (ACTIVATE Identity with per-partition scale+bias APs, deferred
        past exp(g10) in queue order); residual add on GpSimd (DVE for
        tile 7); out DMA split across sync/gpsimd queues. Tile 7's
        output projection is split into two 256-col halves so the first
        half runs right after the final mm2 burst.
       (b_g is folded into bn_shift' on the host: w_out @ b_g is constant)

Prologue: x loads stream as 16 x 128KB per-block DMAs split across the
sync and gpsimd queues (single-queue load serialized 11us before the
first matmul), consts on the scalar HWDGE queue ordered by first use;
theta/phi/gt projections pipeline per 512-col block, with each block's
DMA triggers emitted interleaved one block ahead of the consuming
matmuls (emitting all loads upfront stalled the sync HWDGE ring at 4
in-flight and put the first matmul behind a coarse ~4.5us semaphore).
Iteration 0's first exp groups are split across both engines to halve
exp latency during pipeline fill (no pv work exists yet to hide it).

PSUM (8 banks): S^T groups of 3 chunks [128,1536] double-buffered (6 banks)
+ a 2-slot ring where two [128,129] y-accumulators pack into one bank (the
second starts with start=False onto the bank cleared by the first's
start=True) and the output-projection tiles reuse the same slots.
"""
import os
import sys

sys.path.insert(0, "/opt/trn_rl_repo")

import numpy as np
import ml_dtypes

import concourse.bass as bass
import concourse.bacc as bacc
import concourse.mybir as mybir
import concourse.tile as tile
from concourse.tile import add_dep_helper
from concourse.bass_utils import run_bass_kernel_spmd

F32 = mybir.dt.float32
F16 = mybir.dt.float16
BF16 = mybir.dt.bfloat16
I16 = mybir.dt.int16
ADD = mybir.AluOpType.add
MULT = mybir.AluOpType.mult
EXP = mybir.ActivationFunctionType.Exp
IDENT = mybir.ActivationFunctionType.Identity

B, C, O, N = 8, 256, 128, 4096
NT = 512
N_TILES = N // NT            # 8
M_CHUNKS = N // 128          # 32
# S^T groups: chunks per exp ACTIVATE (3 banks => double-buffered in 6)
GROUPS = [(c, min(c + 3, M_CHUNKS)) for c in range(0, M_CHUNKS, 3)]  # 11 groups
# groups whose exp runs as a DVE fast-exp instead of a ScalarE ACTIVATE
DVE_GROUPS = {1, 3, 5, 7, 10}
# Schraudolph fast-exp into bf16 bits: i16 = trunc(S*A + Bc); bits are bf16.
# c=0.0436 calibrated for truncation semantics; validated rel err 0.007.
FEXP_A = float(128.0 / np.log(2.0))
FEXP_B = float(128.0 * (127.0 - 0.0436))
BN_EPS = 1e-5


def build_nc():
    nc = bacc.Bacc()

    xin = nc.dram_tensor("xin", [C, N], F16, kind="ExternalInput")
    wth = nc.dram_tensor("wth", [C, O], F16, kind="ExternalInput")     # w_theta.T
    wph = nc.dram_tensor("wph", [C, O], F16, kind="ExternalInput")     # w_phi.T
    wg = nc.dram_tensor("wg", [C, O], F16, kind="ExternalInput")       # w_g.T
    wout = nc.dram_tensor("wout", [O, C], BF16, kind="ExternalInput")  # w_out.T
    bth = nc.dram_tensor("bth", [O, 1], F32, kind="ExternalInput")
    bph = nc.dram_tensor("bph", [O, 1], F32, kind="ExternalInput")
    identm = nc.dram_tensor("identm", [128, 128], BF16, kind="ExternalInput")
    bnscale = nc.dram_tensor("bnscale", [128, 2], F32, kind="ExternalInput")
    bnshift = nc.dram_tensor("bnshift", [128, 2], F32, kind="ExternalInput")
    out = nc.dram_tensor("out", [C, N], F32, kind="ExternalOutput")

    with tile.TileContext(nc) as tc:
        with tc.tile_pool(name="const", bufs=1) as const, \
             tc.tile_pool(name="xpool", bufs=1) as xpool, \
             tc.tile_pool(name="proj", bufs=1) as proj, \
             tc.tile_pool(name="yt_pool", bufs=3) as yt_pool, \
             tc.tile_pool(name="small", bufs=6) as small, \
             tc.tile_pool(name="ostage", bufs=6) as ostage:

            # ---- constants on the scalar HWDGE queue (each DMA has ~600ns
            # fixed queue occupancy; keeping them off sync/gpsimd lets the x
            # blocks start streaming immediately) ----
            wth_sb = const.tile([128, 2 * O], F16)
            wph_sb = const.tile([128, 2 * O], F16)
            wg_sb = const.tile([128, 2 * O], F16)
            bth_sb = const.tile([O, 1], F32)
            bph_sb = const.tile([O, 1], F32)
            for k in range(2):
                csl = slice(k * 128, (k + 1) * 128)
                osl = slice(k * O, (k + 1) * O)
                nc.scalar.dma_start(wth_sb[:, osl], wth[csl, :])
                nc.scalar.dma_start(wg_sb[:, osl], wg[csl, :])
                nc.scalar.dma_start(wph_sb[:, osl], wph[csl, :])
            nc.scalar.dma_start(bth_sb[:], bth[:])
            nc.scalar.dma_start(bph_sb[:], bph[:])
            wout_sb = const.tile([O, C], BF16)
            nc.scalar.dma_start(wout_sb[:], wout[:])
            bnscale_sb = const.tile([128, 2], F32)
            bnshift_sb = const.tile([128, 2], F32)
            nc.scalar.dma_start(bnscale_sb[:], bnscale[:])
            nc.scalar.dma_start(bnshift_sb[:], bnshift[:])
            ident_sb = const.tile([128, 128], BF16)
            nc.scalar.dma_start(ident_sb[:], identm[:])

            # ---- x load: one tile per (half, 512-col block) so consumers
            # depend on exactly the DMA they need, halves on 2 queues ----
            x_blk = [[xpool.tile([128, NT], F16, name=f"x{k}_{b}")
                      for b in range(N_TILES)] for k in range(2)]

            theta_h = proj.tile([O, N], F16)
            phi_h = proj.tile([O, N], F16)
            gt_sb = proj.tile([128, M_CHUNKS * (O + 1)], BF16)
            # preload the EXP activation table before the first real exp
            exp_warm = small.tile([O, 1], F32)
            nc.scalar.activation(exp_warm[:], bth_sb[:], EXP)

            # ---- projections, pipelined per 512-col block; DMA triggers are
            # emitted interleaved with the consuming matmuls (one block ahead)
            # so completion semaphores bind per block - emitting all 16 loads
            # upfront made the first matmul wait ~4.5us on a coarse semaphore
            # and stalled the sync HWDGE ring at 4 in-flight ----
            # blocks 6-7 project through a pool placed on banks 6-7 (the
            # future ya banks, unused until iteration 0's first y write) so
            # the S^T ring's banks 0-5 free as soon as block 5 drains -
            # otherwise the first S^T waits ~3us for the last blocks' bias
            # and gt-copy reads
            with tc.tile_pool(name="thph_ps", bufs=2, space="PSUM") as thph_ps, \
                 tc.tile_pool(name="gt_ps", bufs=2, space="PSUM") as gt_ps, \
                 tc.tile_pool(name="thph2_ps", bufs=1, space="PSUM") as thph2_ps:
                for b in range(-1, N_TILES):
                    if b + 1 < N_TILES:
                        nbsl = slice((b + 1) * NT, (b + 2) * NT)
                        nc.sync.dma_start(x_blk[0][b + 1][:], xin[0:128, nbsl])
                        nc.gpsimd.dma_start(x_blk[1][b + 1][:], xin[128:256, nbsl])
                    if b < 0:
                        continue
                    bsl = slice(b * NT, (b + 1) * NT)
                    if b < 6:
                        pp = thph_ps.tile([128, 2 * NT], F32, name="pp")
                        pth, pph = pp[:, 0:NT], pp[:, NT:2 * NT]
                        for k in range(2):
                            nc.tensor.matmul(
                                pth, wth_sb[:, k * O:(k + 1) * O],
                                x_blk[k][b][:], start=(k == 0), stop=(k == 1),
                            )
                        for k in range(2):
                            nc.tensor.matmul(
                                pph, wph_sb[:, k * O:(k + 1) * O],
                                x_blk[k][b][:], start=(k == 0), stop=(k == 1),
                            )
                        if b % 2 == 0:
                            nc.scalar.activation(theta_h[:, bsl], pth, IDENT, bias=bth_sb[:])
                            nc.vector.tensor_scalar_add(phi_h[:, bsl], pph, bph_sb[:])
                        else:
                            nc.vector.tensor_scalar_add(theta_h[:, bsl], pth, bth_sb[:])
                            nc.scalar.activation(phi_h[:, bsl], pph, IDENT, bias=bph_sb[:])
                        for c in range(4 * b, 4 * b + 4):
                            gp = gt_ps.tile([128, O], F32, name="gp")
                            q = (c % 4) * 128
                            for k in range(2):
                                nc.tensor.matmul(
                                    gp[:],
                                    x_blk[k][b][:, q:q + 128],
                                    wg_sb[:, k * O:(k + 1) * O],
                                    start=(k == 0), stop=(k == 1),
                                )
                            base = c * (O + 1)
                            if c % 2 == 0:
                                nc.vector.tensor_copy(gt_sb[:, base:base + O], gp[:])
                            else:
                                nc.scalar.copy(gt_sb[:, base:base + O], gp[:])
                            nc.gpsimd.memset(
                                gt_sb[:, base + O:base + O + 1], 1.0)
                    else:
                        # serialized through the single borrowed bank; every
                        # consumer here has multi-us slack (phi chunks 24-31
                        # are first read at S^T position 8 of iteration 0,
                        # gt chunks 24-31 by its mm2 position 10)
                        pth = thph2_ps.tile([128, NT], F32, name="pp2")
                        for k in range(2):
                            nc.tensor.matmul(
                                pth[:], wth_sb[:, k * O:(k + 1) * O],
                                x_blk[k][b][:], start=(k == 0), stop=(k == 1),
                            )
                        nc.scalar.activation(theta_h[:, bsl], pth[:], IDENT, bias=bth_sb[:])
                        pph = thph2_ps.tile([128, NT], F32, name="pp2")
                        for k in range(2):
                            nc.tensor.matmul(
                                pph[:], wph_sb[:, k * O:(k + 1) * O],
                                x_blk[k][b][:], start=(k == 0), stop=(k == 1),
                            )
                        nc.vector.tensor_scalar_add(phi_h[:, bsl], pph[:], bph_sb[:])
                        ppg = thph2_ps.tile([128, NT], F32, name="pp2")
                        first_i = None
                        for idx, c in enumerate(range(4 * b, 4 * b + 4)):
                            q = (c % 4) * 128
                            for k in range(2):
                                i0 = nc.tensor.matmul(
                                    ppg[:, idx * 128:(idx + 1) * 128],
                                    x_blk[k][b][:, q:q + 128],
                                    wg_sb[:, k * O:(k + 1) * O],
                                    start=(idx == 0 and k == 0),
                                    stop=(idx == 3 and k == 1),
                                    skip_group_check=True,
                                )
                                if idx == 0 and k == 0:
                                    first_i = i0
                                elif k == 0:
                                    add_dep_helper(i0.ins, first_i.ins, sync=False,
                                                   reason="bank-pack: clear before write")
                        for idx, c in enumerate(range(4 * b, 4 * b + 4)):
                            base = c * (O + 1)
                            if c % 2 == 0:
                                nc.vector.tensor_copy(
                                    gt_sb[:, base:base + O],
                                    ppg[:, idx * 128:(idx + 1) * 128])
                            else:
                                nc.scalar.copy(
                                    gt_sb[:, base:base + O],
                                    ppg[:, idx * 128:(idx + 1) * 128])
                            nc.gpsimd.memset(
                                gt_sb[:, base + O:base + O + 1], 1.0)

            # ---- software-pipelined attention ----
            # Iteration it: S^T+exp+y01(s0,s1) mm2 for tile nt=it, with tile
            # pv=it-1's y23(s2,s3) mm2 FRONT-LOADED into positions 0..5 (its
            # P^T tiles are all ready), its norms+transposes at position 6 and
            # its output projection at position 9 of the same iteration. This
            # keeps the y-accumulator PSUM banks a full half-iteration ahead
            # of their next writer (the old layout stalled the PE ~1.4us at
            # every tile boundary waiting for end-of-tile norms, then paid
            # ~3us of half-clock p-state ramp).
            with tc.tile_pool(name="pt_pool", bufs=22) as pt_pool, \
                 tc.tile_pool(name="st_ps", bufs=2, space="PSUM") as st_ps, \
                 tc.tile_pool(name="ya", bufs=2, space="PSUM") as ya:

                def mm2(y, pt, s, c0, c1, pt_c0):
                    # y accumulator slice gets chunks [c0, c1) of P^T tile pt.
                    # Two accumulators share one PSUM bank: only the first
                    # (s even) opens the group (start=True clears the whole
                    # bank); the second writes start=False onto cleared bits.
                    first = None
                    for c in range(c0, c1):
                        i = nc.tensor.matmul(
                            y[:],
                            pt[:, (c - pt_c0) * NT + s * 128:(c - pt_c0) * NT + (s + 1) * 128],
                            gt_sb[:, c * (O + 1):(c + 1) * (O + 1)],
                            start=(c == 0 and s % 2 == 0),
                            stop=(c == M_CHUNKS - 1),
                            skip_group_check=True,
                        )
                        if first is None:
                            first = i
                    return first

                def norm_transpose(y, yt_tile, col, q):
                    recip = small.tile([128, 1], F32)
                    nc.vector.reciprocal(recip[:], y[:, O:O + 1])
                    ynorm = small.tile([128, O], BF16)
                    nc.vector.tensor_scalar_mul(ynorm[:], y[:, 0:O], recip[:])
                    q.dma_start_transpose(yt_tile[:, col:col + 128], ynorm[:])

                def norm_pair_pe(ytile, yt_tile, colbase, copy_eng):
                    # last-tile path: transpose on the (idle) PE instead of the
                    # DMA xbar, staging through the spare tail of the y PSUM
                    # bank; the serialized DMA transposes were ~5us of pure
                    # tail latency. start=True zeroes the whole bank, so the
                    # first transpose must wait for BOTH slices' norm reads.
                    yn = []
                    for lo in (0, 130):
                        recip = small.tile([128, 1], F32)
                        nc.vector.reciprocal(recip[:], ytile[:, lo + O:lo + O + 1])
                        ynorm = small.tile([128, O], BF16)
                        mi = nc.vector.tensor_scalar_mul(
                            ynorm[:], ytile[:, lo:lo + O], recip[:])
                        yn.append((ynorm, mi))
                    dstb = ytile[:, 260:392].bitcast(BF16)
                    prev = None
                    for j, (ynorm, mi) in enumerate(yn):
                        dst = dstb[:, j * 132:j * 132 + 128]
                        im = nc.tensor.matmul(
                            dst, ynorm[:], ident_sb[:], is_transpose=True,
                            start=(j == 0), stop=True, skip_group_check=True,
                        )
                        if j == 0:
                            add_dep_helper(im.ins, yn[1][1].ins, sync=True,
                                           reason="bank clear only after all norm reads")
                        else:
                            add_dep_helper(im.ins, prev.ins, sync=False,
                                           reason="bank-pack: clear before second write")
                        prev = im
                        if copy_eng is nc.scalar:
                            nc.scalar.copy(
                                yt_tile[:, colbase + j * 128:colbase + (j + 1) * 128], dst)
                        else:
                            copy_eng.tensor_copy(
                                yt_tile[:, colbase + j * 128:colbase + (j + 1) * 128], dst)

                def bn_res_dma(src_ap, ct, t, off, width):
                    # bn on ScalarE: Identity(z*scale + bias) with per-partition
                    # APs; residual add on GpSimd (DVE for the last tile, whose
                    # tail has DVE idle); out DMA split across queues.
                    obn = ostage.tile([128, width], F32)
                    nc.scalar.activation(
                        obn[:], src_ap, IDENT,
                        bias=bnshift_sb[:, ct:ct + 1], scale=bnscale_sb[:, ct:ct + 1],
                    )
                    ores = ostage.tile([128, width], F32, name="ores")
                    eng = nc.vector if t == N_TILES - 1 else nc.gpsimd
                    eng.tensor_tensor(
                        ores[:], obn[:], x_blk[ct][t][:, off:off + width], op=ADD
                    )
                    # last tile: keep everything on sync so the gpsimd DMA
                    # ring has nothing late to drain in the epilogue
                    q = nc.sync if (ct == 0 or t == N_TILES - 1) else nc.gpsimd
                    q.dma_start(
                        out[ct * 128:(ct + 1) * 128, t * NT + off:t * NT + off + width],
                        ores[:])

                def out_proj_mm_full(t, yt_tile):
                    o01 = st_ps.tile([128, 1024], F32, name="o01", tag="st")
                    for ct in range(2):
                        nc.tensor.matmul(
                            o01[:, ct * NT:(ct + 1) * NT],
                            wout_sb[:, ct * 128:(ct + 1) * 128],
                            yt_tile[:],
                            start=True, stop=True,
                            skip_group_check=True,
                        )
                    return [(o01[:, ct * NT:(ct + 1) * NT], ct, t, 0, NT)
                            for ct in range(2)]

                def out_proj_mm_half(t, h, yt_tile):
                    oh = st_ps.tile([128, 512], F32, name="oh", tag="st")
                    csl = slice(h * 256, (h + 1) * 256)
                    for ct in range(2):
                        nc.tensor.matmul(
                            oh[:, ct * 256:(ct + 1) * 256],
                            wout_sb[:, ct * 128:(ct + 1) * 128],
                            yt_tile[:, csl],
                            start=True, stop=True,
                            skip_group_check=True,
                        )
                    return [(oh[:, ct * 256:(ct + 1) * 256], ct, t, h * 256, 256)
                            for ct in range(2)]

                def y23_groups(pos):
                    # front-load the 11 groups of the previous tile's y23 mm2
                    # into positions 0..5 (their P^T tiles are all ready)
                    if pos < 5:
                        return [2 * pos, 2 * pos + 1]
                    if pos == 5:
                        return [10]
                    return []

                pts_prev = None
                yts = {}
                n_grps = len(GROUPS)
                last = N_TILES - 1
                for it in range(N_TILES + 1):
                    nt = it if it < N_TILES else None
                    pv = it - 1 if it >= 1 else None
                    if nt is not None:
                        ntsl = slice(nt * NT, (nt + 1) * NT)
                        y01 = ya.tile([128, 392], F32, name="y01", tag="ya")
                        yts[nt] = yt_pool.tile([O, NT], BF16, name=f"yt{nt}")
                    if pv is not None:
                        y23 = ya.tile([128, 392], F32, name="y23", tag="ya")
                    pts_cur = []
                    bns = []
                    for g in range(n_grps):
                        if pv is not None and g == 0:
                            # position 0: previous tile's y23 mm2 first (deps
                            # long met) so the PE has work while the previous
                            # out-projection's bn reads free the S^T-ring slot
                            for gg in y23_groups(0):
                                c0, c1 = GROUPS[gg]
                                ia = mm2(y23[:, 0:O + 1], pts_prev[gg], 2, c0, c1, c0)
                                ib = mm2(y23[:, 130:259], pts_prev[gg], 3, c0, c1, c0)
                                if c0 == 0:
                                    add_dep_helper(ib.ins, ia.ins, sync=False,
                                                   reason="bank-pack: clear before first write")
                        if nt is not None:
                            c0, c1 = GROUPS[g]
                            w = (c1 - c0) * NT
                            st = st_ps.tile([128, 3 * NT], F32, name="st", tag="st")
                            for c in range(c0, c1):
                                nc.tensor.matmul(
                                    st[:, (c - c0) * NT:(c - c0 + 1) * NT],
                                    phi_h[:, c * 128:(c + 1) * 128],
                                    theta_h[:, ntsl],
                                    start=True, stop=True,
                                )
                            pt = pt_pool.tile([128, 3 * NT], BF16)
                            if it == 0 and g < 4:
                                # pipeline fill: no pv work exists to hide a
                                # whole-group exp latency, so halve it by
                                # splitting the group across both engines
                                nc.scalar.activation(pt[:, :2 * NT], st[:, :2 * NT], EXP)
                                nc.vector.tensor_scalar(
                                    pt[:, 2 * NT:w].bitcast(I16), st[:, 2 * NT:w],
                                    FEXP_A, FEXP_B, op0=MULT, op1=ADD,
                                )
                            elif g in DVE_GROUPS:
                                nc.vector.tensor_scalar(
                                    pt[:, :w].bitcast(I16), st[:, :w],
                                    FEXP_A, FEXP_B, op0=MULT, op1=ADD,
                                )
                            else:
                                nc.scalar.activation(pt[:, :w], st[:, :w], EXP)
                            pts_cur.append(pt)
                            if g > 1:
                                # y01 mm2 lags TWO groups behind S^T: exp(g)
                                # then has ~2 pipeline positions before its
                                # consumer instead of 1 (a 3-chunk exp takes
                                # 1.4-1.7us but a back-half position is ~1us)
                                pc0, pc1 = GROUPS[g - 2]
                                ia = mm2(y01[:, 0:O + 1], pts_cur[g - 2], 0, pc0, pc1, pc0)
                                ib = mm2(y01[:, 130:259], pts_cur[g - 2], 1, pc0, pc1, pc0)
                                if pc0 == 0:
                                    add_dep_helper(ib.ins, ia.ins, sync=False,
                                                   reason="bank-pack: clear before first write")
                        if pv is not None and g > 0:
                            for gg in y23_groups(g):
                                c0, c1 = GROUPS[gg]
                                ia = mm2(y23[:, 0:O + 1], pts_prev[gg], 2, c0, c1, c0)
                                ib = mm2(y23[:, 130:259], pts_prev[gg], 3, c0, c1, c0)
                                if c0 == 0:
                                    add_dep_helper(ib.ins, ia.ins, sync=False,
                                                   reason="bank-pack: clear before first write")
                        if pv is not None and g == 6:
                            if pv == last:
                                # tail: first half of tile 7's output projection
                                # (yt cols 0:256 transposed during the previous
                                # iteration) straight after the final mm2 burst
                                bns += out_proj_mm_half(pv, 0, yts[pv])
                                norm_pair_pe(y23, yts[pv], 256, nc.scalar)
                            else:
                                norm_transpose(y23[:, 0:O + 1], yts[pv], 256, nc.sync)
                                norm_transpose(y23[:, 130:259], yts[pv], 384, nc.sync)
                        if pv is not None and g == 10:
                            if pv < last:
                                bns += out_proj_mm_full(pv, yts[pv])
                            else:
                                bns += out_proj_mm_half(pv, 1, yts[pv])
                    if nt is not None:
                        for gg in (n_grps - 2, n_grps - 1):
                            c0, c1 = GROUPS[gg]
                            mm2(y01[:, 0:O + 1], pts_cur[gg], 0, c0, c1, c0)
                            mm2(y01[:, 130:259], pts_cur[gg], 1, c0, c1, c0)
                    for args in bns:
                        bn_res_dma(*args)
                    if nt is not None:
                        if nt == last:
                            # copies on DVE: scalar is still draining exp/bn
                            # residue here, and these copies gate h0
                            norm_pair_pe(y01, yts[nt], 0, nc.vector)
                        else:
                            norm_transpose(y01[:, 0:O + 1], yts[nt], 0, nc.sync)
                            norm_transpose(y01[:, 130:259], yts[nt], 128, nc.sync)
                    pts_prev = pts_cur

    nc.finalize()
    return nc


_NC_CACHE = None


def _get_nc():
    global _NC_CACHE
    if _NC_CACHE is None:
        _NC_CACHE = build_nc()
    return _NC_CACHE


def _prepare_in_maps(inputs):
    x = np.ascontiguousarray(np.asarray(inputs["x"], dtype=np.float32)).reshape(B, C, N)
    xh = x.astype(np.float16)
    wth = np.ascontiguousarray(np.asarray(inputs["w_theta"], np.float32).T).astype(np.float16)
    wph = np.ascontiguousarray(np.asarray(inputs["w_phi"], np.float32).T).astype(np.float16)
    wg = np.ascontiguousarray(np.asarray(inputs["w_g"], np.float32).T).astype(np.float16)
    w_out = np.asarray(inputs["w_out"], np.float32)
    wout = np.ascontiguousarray(w_out.T).astype(ml_dtypes.bfloat16)
    bth = np.asarray(inputs["b_theta"], np.float32).reshape(O, 1)
    bph = np.asarray(inputs["b_phi"], np.float32).reshape(O, 1)
    inv = np.asarray(inputs["bn_gamma"], np.float32) / np.sqrt(
        np.asarray(inputs["bn_var"], np.float32) + BN_EPS)
    shift = (np.asarray(inputs["b_out"], np.float32) * inv
             + np.asarray(inputs["bn_beta"], np.float32)
             - np.asarray(inputs["bn_mean"], np.float32) * inv)
    # fold the g-branch bias through the output projection: softmax rows sum
    # to 1, so attn @ (g + b_g) = attn @ g + b_g, and w_out @ b_g is constant
    wob = wout.astype(np.float32) .T @ np.asarray(inputs["b_g"], np.float32)
    shift = shift + inv * wob
    bnscale = np.ascontiguousarray(inv.reshape(2, 128).T)
    bnshift = np.ascontiguousarray(shift.reshape(2, 128).T)

    shared = dict(wth=wth, wph=wph, wg=wg, wout=wout, bth=bth, bph=bph,
                  bnscale=bnscale, bnshift=bnshift,
                  identm=np.eye(128, dtype=ml_dtypes.bfloat16))
    return [dict(shared, xin=np.ascontiguousarray(xh[b])) for b in range(B)]


def _install_ntff_shim():
    """This image's antenv lacks axon_hooks; provide it from trn_boot's
    ctypes implementation so trace=True can capture NTFF profiles."""
    import types
    try:
        import antenv.axon_hooks  # noqa: F401
        return
    except ImportError:
        pass
    if "/root/.axon_site" not in sys.path:
        sys.path.insert(0, "/root/.axon_site")
    from trn_agent_boot.trn_boot import _ntff_profile_via_ctypes
    hook = _ntff_profile_via_ctypes("/opt/axon/libaxon_pjrt.so")
    m = types.ModuleType("antenv.axon_hooks")
    m.get_axon_ntff_profile_hook = lambda: hook
    m.set_axon_ntff_profile_hook = lambda h: None
    sys.modules["antenv.axon_hooks"] = m


def run(inputs, trace=False):
    if trace:
        _install_ntff_shim()
    nc = _get_nc()
    in_maps = _prepare_in_maps(inputs)
    res = run_bass_kernel_spmd(nc, in_maps, list(range(B)), trace=trace)
    outs = np.stack([res.results[b]["out"] for b in range(B)])
    return outs.reshape(B, C, 64, 64), res


def kernel(**inputs) -> np.ndarray:
    out, _ = run(inputs)
    return out


if __name__ == "__main__":
    # quick CoreSim check of one core
    from concourse import bass_interp
    rng = np.random.default_rng(0)
    fake = {
        "x": rng.standard_normal((B, C, 64, 64)).astype(np.float32),
        "w_theta": (rng.standard_normal((O, C)) * 0.05).astype(np.float32),
        "b_theta": (rng.standard_normal(O) * 0.05).astype(np.float32),
        "w_phi": (rng.standard_normal((O, C)) * 0.05).astype(np.float32),
        "b_phi": (rng.standard_normal(O) * 0.05).astype(np.float32),
        "w_g": (rng.standard_normal((O, C)) * 0.05).astype(np.float32),
        "b_g": (rng.standard_normal(O) * 0.05).astype(np.float32),
        "w_out": (rng.standard_normal((C, O)) * 0.05).astype(np.float32),
        "b_out": (rng.standard_normal(C) * 0.05).astype(np.float32),
        "bn_gamma": rng.standard_normal(C).astype(np.float32),
        "bn_beta": rng.standard_normal(C).astype(np.float32),
        "bn_mean": rng.standard_normal(C).astype(np.float32),
        "bn_var": rng.uniform(0.5, 1.5, C).astype(np.float32),
    }
    nc = _get_nc()
    in_maps = _prepare_in_maps(fake)
    sim = bass_interp.CoreSim(nc)
    for k, v in in_maps[0].items():
        sim.tensor(k)[:] = v
    sim.simulate()
    got = np.asarray(sim.tensor("out"))

    x0 = fake["x"][0].reshape(C, N)
    th = fake["w_theta"] @ x0 + fake["b_theta"][:, None]
    ph = fake["w_phi"] @ x0 + fake["b_phi"][:, None]
    gg = fake["w_g"] @ x0 + fake["b_g"][:, None]
    s = th.T @ ph
    p = np.exp(s - s.max(1, keepdims=True))
    a = p / p.sum(1, keepdims=True)
    yy = a @ gg.T
    wy = fake["w_out"] @ yy.T + fake["b_out"][:, None]
    inv = fake["bn_gamma"] / np.sqrt(fake["bn_var"] + BN_EPS)
    bn = wy * inv[:, None] + (fake["bn_beta"] - fake["bn_mean"] * inv)[:, None]
    want = x0 + bn
    err = np.abs(got - want).max()
    print("CoreSim absmax err:", err, "rel:", err / np.abs(want).max())


# revision 44
# speedup vs baseline: 1.0184x; 1.0135x over previous
"""NonLocalBlock (B=8, C=256, HW=64x64) Trainium2 kernel.

Data-parallel over batch: one sample per NeuronCore (8 cores).
Per core (everything on-chip; the [N,N]=67MB attention matrix never
touches HBM):

  x [C=256, N=4096] fp16 --DMA--> SBUF (serves logit path, g path, residual)
  theta = w_theta @ x + b_theta    [O=128, N]  fp16   (PE fp16)
  phi   = w_phi   @ x + b_phi      [O=128, N]  fp16
  gt    = (w_g @ x)^T              [N, O] bf16 chunks + ones column
  per 512-wide n-tile (iteration it handles tile nt=it plus leftovers of
  tile pv=it-1):
    S^T chunks [m=128, n=512] = phi_chunk^T . theta_tile  (PE fp16 -> PSUM)
    P^T = exp(S^T): 3-chunk groups alternate between ScalarE (exact EXP
        ACTIVATE, PSUM->SBUF bf16) and DVE (Schraudolph fast-exp: one
        tensor_scalar mult-add producing int16 bits that ARE the bf16
        exp approximation, ~3% rel err - calibrated against the 2e-2
        gate). Splitting exp across two engines removes ScalarE as the
        pipeline pacer (exp is 109us of ScalarE work if unsplit).
    y[s] [128, 129] += P^T_chunk^T . [gt_chunk | ones]    (PE bf16)
        col 128 accumulates the softmax row-sum for free.
        Slices s=0,1 of tile nt run in iteration it lagging the S^T
        pipeline by TWO groups (so a 1.4-1.7us exp has ~2 pipeline
        positions of slack, not 1); slices s=2,3 of tile pv are
        front-loaded into positions 0..5 (all their P^T is ready), so
        the y-accumulator banks recycle half an iteration before their
        next writer - no PE stall + p-state ramp at tile boundaries.
    ynorm = y[:, :128] * (1/y[:,128])    (DVE)
    yt[o, n-sub] = DMA-xbar-transpose(ynorm) on the sync queue; tile 7
        instead transposes on the then-idle PE (is_transpose matmul vs
        an identity, staged through the spare tail of the y PSUM bank)
        - the 4 serialized DMA transposes were ~5us of pure tail latency
    z = w_out^T . yt at position 10 of iteration pv+1; bn on ScalarE
        (ACTIVATE Identity with per-partition scale+bias APs, deferred
        past exp(g10) in queue order); residual add on GpSimd (DVE for
        tile 7); out DMA split across sync/gpsimd queues. Tile 7's
        output projection is split into two 256-col halves so the first
        half runs right after the final mm2 burst.
       (b_g is folded into bn_shift' on the host: w_out @ b_g is constant)

Prologue: x loads stream as 16 x 128KB per-block DMAs split across the
sync and gpsimd queues (single-queue load serialized 11us before the
first matmul), consts on the scalar HWDGE queue ordered by first use;
theta/phi/gt projections pipeline per 512-col block, with each block's
DMA triggers emitted interleaved one block ahead of the consuming
matmuls (emitting all loads upfront stalled the sync HWDGE ring at 4
in-flight and put the first matmul behind a coarse ~4.5us semaphore).
Iteration 0's first exp groups are split across both engines to halve
exp latency during pipeline fill (no pv work exists yet to hide it).
Blocks 6-7 project through a single bank borrowed from the (still
unused) ya region so the S^T ring's six banks free as soon as block 5
drains instead of waiting ~3us for the last blocks' bias/copy reads.

PSUM (8 banks): S^T groups of 3 chunks [128,1536] double-buffered (6 banks)
+ a 2-slot ring where two [128,129] y-accumulators pack into one bank (the
second starts with start=False onto the bank cleared by the first's
start=True) and the output-projection tiles reuse the same slots.
"""
import os
import sys

sys.path.insert(0, "/opt/trn_rl_repo")

import numpy as np
import ml_dtypes

import concourse.bass as bass
import concourse.bacc as bacc
import concourse.mybir as mybir
import concourse.tile as tile
from concourse.tile import add_dep_helper
from concourse.bass_utils import run_bass_kernel_spmd

F32 = mybir.dt.float32
F16 = mybir.dt.float16
BF16 = mybir.dt.bfloat16
I16 = mybir.dt.int16
ADD = mybir.AluOpType.add
MULT = mybir.AluOpType.mult
EXP = mybir.ActivationFunctionType.Exp
IDENT = mybir.ActivationFunctionType.Identity

B, C, O, N = 8, 256, 128, 4096
NT = 512
N_TILES = N // NT            # 8
M_CHUNKS = N // 128          # 32
# S^T groups: chunks per exp ACTIVATE (3 banks => double-buffered in 6)
GROUPS = [(c, min(c + 3, M_CHUNKS)) for c in range(0, M_CHUNKS, 3)]  # 11 groups
# groups whose exp runs as a DVE fast-exp instead of a ScalarE ACTIVATE
DVE_GROUPS = {1, 3, 5, 7, 10}
# Schraudolph fast-exp into bf16 bits: i16 = trunc(S*A + Bc); bits are bf16.
# c=0.0436 calibrated for truncation semantics; validated rel err 0.007.
FEXP_A = float(128.0 / np.log(2.0))
FEXP_B = float(128.0 * (127.0 - 0.0436))
BN_EPS = 1e-5


def build_nc():
    nc = bacc.Bacc()

    xin = nc.dram_tensor("xin", [C, N], F16, kind="ExternalInput")
    wth = nc.dram_tensor("wth", [C, O], F16, kind="ExternalInput")     # w_theta.T
    wph = nc.dram_tensor("wph", [C, O], F16, kind="ExternalInput")     # w_phi.T
    wg = nc.dram_tensor("wg", [C, O], F16, kind="ExternalInput")       # w_g.T
    wout = nc.dram_tensor("wout", [O, C], BF16, kind="ExternalInput")  # w_out.T
    bth = nc.dram_tensor("bth", [O, 1], F32, kind="ExternalInput")
    bph = nc.dram_tensor("bph", [O, 1], F32, kind="ExternalInput")
    identm = nc.dram_tensor("identm", [128, 128], BF16, kind="ExternalInput")
    bnscale = nc.dram_tensor("bnscale", [128, 2], F32, kind="ExternalInput")
    bnshift = nc.dram_tensor("bnshift", [128, 2], F32, kind="ExternalInput")
    out = nc.dram_tensor("out", [C, N], F32, kind="ExternalOutput")

    with tile.TileContext(nc) as tc:
        with tc.tile_pool(name="const", bufs=1) as const, \
             tc.tile_pool(name="xpool", bufs=1) as xpool, \
             tc.tile_pool(name="proj", bufs=1) as proj, \
             tc.tile_pool(name="yt_pool", bufs=3) as yt_pool, \
             tc.tile_pool(name="small", bufs=6) as small, \
             tc.tile_pool(name="ostage", bufs=6) as ostage:

            # ---- constants on the scalar HWDGE queue (each DMA has ~600ns
            # fixed queue occupancy; keeping them off sync/gpsimd lets the x
            # blocks start streaming immediately) ----
            wth_sb = const.tile([128, 2 * O], F16)
            wph_sb = const.tile([128, 2 * O], F16)
            wg_sb = const.tile([128, 2 * O], F16)
            bth_sb = const.tile([O, 1], F32)
            bph_sb = const.tile([O, 1], F32)
            for k in range(2):
                csl = slice(k * 128, (k + 1) * 128)
                osl = slice(k * O, (k + 1) * O)
                nc.scalar.dma_start(wth_sb[:, osl], wth[csl, :])
                nc.scalar.dma_start(wg_sb[:, osl], wg[csl, :])
                nc.scalar.dma_start(wph_sb[:, osl], wph[csl, :])
            nc.scalar.dma_start(bth_sb[:], bth[:])
            nc.scalar.dma_start(bph_sb[:], bph[:])
            wout_sb = const.tile([O, C], BF16)
            nc.scalar.dma_start(wout_sb[:], wout[:])
            bnscale_sb = const.tile([128, 2], F32)
            bnshift_sb = const.tile([128, 2], F32)
            nc.scalar.dma_start(bnscale_sb[:], bnscale[:])
            nc.scalar.dma_start(bnshift_sb[:], bnshift[:])
            ident_sb = const.tile([128, 128], BF16)
            nc.scalar.dma_start(ident_sb[:], identm[:])

            # ---- x load: one tile per (half, 512-col block) so consumers
            # depend on exactly the DMA they need, halves on 2 queues ----
            x_blk = [[xpool.tile([128, NT], F16, name=f"x{k}_{b}")
                      for b in range(N_TILES)] for k in range(2)]

            theta_h = proj.tile([O, N], F16)
            phi_h = proj.tile([O, N], F16)
            gt_sb = proj.tile([128, M_CHUNKS * (O + 1)], BF16)
            # preload the EXP activation table before the first real exp
            exp_warm = small.tile([O, 1], F32)
            nc.scalar.activation(exp_warm[:], bth_sb[:], EXP)

            # ---- projections, pipelined per 512-col block; DMA triggers are
            # emitted interleaved with the consuming matmuls (one block ahead)
            # so completion semaphores bind per block - emitting all 16 loads
            # upfront made the first matmul wait ~4.5us on a coarse semaphore
            # and stalled the sync HWDGE ring at 4 in-flight ----
            # blocks 6-7 project through a pool placed on banks 6-7 (the
            # future ya banks, unused until iteration 0's first y write) so
            # the S^T ring's banks 0-5 free as soon as block 5 drains -
            # otherwise the first S^T waits ~3us for the last blocks' bias
            # and gt-copy reads
            with tc.tile_pool(name="thph_ps", bufs=2, space="PSUM") as thph_ps, \
                 tc.tile_pool(name="gt_ps", bufs=2, space="PSUM") as gt_ps, \
                 tc.tile_pool(name="thph2_ps", bufs=1, space="PSUM") as thph2_ps:
                for b in range(-1, N_TILES):
                    if b + 1 < N_TILES:
                        nbsl = slice((b + 1) * NT, (b + 2) * NT)
                        nc.sync.dma_start(x_blk[0][b + 1][:], xin[0:128, nbsl])
                        nc.gpsimd.dma_start(x_blk[1][b + 1][:], xin[128:256, nbsl])
                    if b < 0:
                        continue
                    bsl = slice(b * NT, (b + 1) * NT)
                    if b < 6:
                        pp = thph_ps.tile([128, 2 * NT], F32, name="pp")
                        pth, pph = pp[:, 0:NT], pp[:, NT:2 * NT]
                        for k in range(2):
                            nc.tensor.matmul(
                                pth, wth_sb[:, k * O:(k + 1) * O],
                                x_blk[k][b][:], start=(k == 0), stop=(k == 1),
                            )
                        for k in range(2):
                            nc.tensor.matmul(
                                pph, wph_sb[:, k * O:(k + 1) * O],
                                x_blk[k][b][:], start=(k == 0), stop=(k == 1),
                            )
                        if b % 2 == 0:
                            nc.scalar.activation(theta_h[:, bsl], pth, IDENT, bias=bth_sb[:])
                            nc.vector.tensor_scalar_add(phi_h[:, bsl], pph, bph_sb[:])
                        else:
                            nc.vector.tensor_scalar_add(theta_h[:, bsl], pth, bth_sb[:])
                            nc.scalar.activation(phi_h[:, bsl], pph, IDENT, bias=bph_sb[:])
                        for c in range(4 * b, 4 * b + 4):
                            gp = gt_ps.tile([128, O], F32, name="gp")
                            q = (c % 4) * 128
                            for k in range(2):
                                nc.tensor.matmul(
                                    gp[:],
                                    x_blk[k][b][:, q:q + 128],
                                    wg_sb[:, k * O:(k + 1) * O],
                                    start=(k == 0), stop=(k == 1),
                                )
                            base = c * (O + 1)
                            if c % 2 == 0:
                                nc.vector.tensor_copy(gt_sb[:, base:base + O], gp[:])
                            else:
                                nc.scalar.copy(gt_sb[:, base:base + O], gp[:])
                            nc.gpsimd.memset(
                                gt_sb[:, base + O:base + O + 1], 1.0)
                    else:
                        # serialized through the single borrowed bank; every
                        # consumer here has multi-us slack (phi chunks 24-31
                        # are first read at S^T position 8 of iteration 0,
                        # gt chunks 24-31 by its mm2 position 10)
                        pth = thph2_ps.tile([128, NT], F32, name="pp2")
                        for k in range(2):
                            nc.tensor.matmul(
                                pth[:], wth_sb[:, k * O:(k + 1) * O],
                                x_blk[k][b][:], start=(k == 0), stop=(k == 1),
                            )
                        nc.scalar.activation(theta_h[:, bsl], pth[:], IDENT, bias=bth_sb[:])
                        pph = thph2_ps.tile([128, NT], F32, name="pp2")
                        for k in range(2):
                            nc.tensor.matmul(
                                pph[:], wph_sb[:, k * O:(k + 1) * O],
                                x_blk[k][b][:], start=(k == 0), stop=(k == 1),
                            )
                        nc.vector.tensor_scalar_add(phi_h[:, bsl], pph[:], bph_sb[:])
                        ppg = thph2_ps.tile([128, NT], F32, name="pp2")
                        first_i = None
                        for idx, c in enumerate(range(4 * b, 4 * b + 4)):
                            q = (c % 4) * 128
                            for k in range(2):
                                i0 = nc.tensor.matmul(
                                    ppg[:, idx * 128:(idx + 1) * 128],
                                    x_blk[k][b][:, q:q + 128],
                                    wg_sb[:, k * O:(k + 1) * O],
                                    start=(idx == 0 and k == 0),
                                    stop=(idx == 3 and k == 1),
                                    skip_group_check=True,
                                )
                                if idx == 0 and k == 0:
                                    first_i = i0
                                elif k == 0:
                                    add_dep_helper(i0.ins, first_i.ins, sync=False,
                                                   reason="bank-pack: clear before write")
                        for idx, c in enumerate(range(4 * b, 4 * b + 4)):
                            base = c * (O + 1)
                            if c % 2 == 0:
                                nc.vector.tensor_copy(
                                    gt_sb[:, base:base + O],
                                    ppg[:, idx * 128:(idx + 1) * 128])
                            else:
                                nc.scalar.copy(
                                    gt_sb[:, base:base + O],
                                    ppg[:, idx * 128:(idx + 1) * 128])
                            nc.gpsimd.memset(
                                gt_sb[:, base + O:base + O + 1], 1.0)

            # ---- software-pipelined attention ----
            # Iteration it: S^T+exp+y01(s0,s1) mm2 for tile nt=it, with tile
            # pv=it-1's y23(s2,s3) mm2 FRONT-LOADED into positions 0..5 (its
            # P^T tiles are all ready), its norms+transposes at position 6 and
            # its output projection at position 9 of the same iteration. This
            # keeps the y-accumulator PSUM banks a full half-iteration ahead
            # of their next writer (the old layout stalled the PE ~1.4us at
            # every tile boundary waiting for end-of-tile norms, then paid
            # ~3us of half-clock p-state ramp).
            with tc.tile_pool(name="pt_pool", bufs=22) as pt_pool, \
                 tc.tile_pool(name="st_ps", bufs=2, space="PSUM") as st_ps, \
                 tc.tile_pool(name="ya", bufs=2, space="PSUM") as ya:

                def mm2(y, pt, s, c0, c1, pt_c0):
                    # y accumulator slice gets chunks [c0, c1) of P^T tile pt.
                    # Two accumulators share one PSUM bank: only the first
                    # (s even) opens the group (start=True clears the whole
                    # bank); the second writes start=False onto cleared bits.
                    first = None
                    for c in range(c0, c1):
                        i = nc.tensor.matmul(
                            y[:],
                            pt[:, (c - pt_c0) * NT + s * 128:(c - pt_c0) * NT + (s + 1) * 128],
                            gt_sb[:, c * (O + 1):(c + 1) * (O + 1)],
                            start=(c == 0 and s % 2 == 0),
                            stop=(c == M_CHUNKS - 1),
                            skip_group_check=True,
                        )
                        if first is None:
                            first = i
                    return first

                def norm_transpose(y, yt_tile, col, q):
                    recip = small.tile([128, 1], F32)
                    nc.vector.reciprocal(recip[:], y[:, O:O + 1])
                    ynorm = small.tile([128, O], BF16)
                    nc.vector.tensor_scalar_mul(ynorm[:], y[:, 0:O], recip[:])
                    q.dma_start_transpose(yt_tile[:, col:col + 128], ynorm[:])

                def norm_pair_pe(ytile, yt_tile, colbase, copy_eng):
                    # last-tile path: transpose on the (idle) PE instead of the
                    # DMA xbar, staging through the spare tail of the y PSUM
                    # bank; the serialized DMA transposes were ~5us of pure
                    # tail latency. start=True zeroes the whole bank, so the
                    # first transpose must wait for BOTH slices' norm reads.
                    yn = []
                    for lo in (0, 130):
                        recip = small.tile([128, 1], F32)
                        nc.vector.reciprocal(recip[:], ytile[:, lo + O:lo + O + 1])
                        ynorm = small.tile([128, O], BF16)
                        mi = nc.vector.tensor_scalar_mul(
                            ynorm[:], ytile[:, lo:lo + O], recip[:])
                        yn.append((ynorm, mi))
                    dstb = ytile[:, 260:392].bitcast(BF16)
                    prev = None
                    for j, (ynorm, mi) in enumerate(yn):
                        dst = dstb[:, j * 132:j * 132 + 128]
                        im = nc.tensor.matmul(
                            dst, ynorm[:], ident_sb[:], is_transpose=True,
                            start=(j == 0), stop=True, skip_group_check=True,
                        )
                        if j == 0:
                            add_dep_helper(im.ins, yn[1][1].ins, sync=True,
                                           reason="bank clear only after all norm reads")
                        else:
                            add_dep_helper(im.ins, prev.ins, sync=False,
                                           reason="bank-pack: clear before second write")
                        prev = im
                        if copy_eng is nc.scalar:
                            nc.scalar.copy(
                                yt_tile[:, colbase + j * 128:colbase + (j + 1) * 128], dst)
                        else:
                            copy_eng.tensor_copy(
                                yt_tile[:, colbase + j * 128:colbase + (j + 1) * 128], dst)

                def bn_res_dma(src_ap, ct, t, off, width):
                    # bn on ScalarE: Identity(z*scale + bias) with per-partition
                    # APs; residual add on GpSimd (DVE for the last tile, whose
                    # tail has DVE idle); out DMA split across queues.
                    obn = ostage.tile([128, width], F32)
                    nc.scalar.activation(
                        obn[:], src_ap, IDENT,
                        bias=bnshift_sb[:, ct:ct + 1], scale=bnscale_sb[:, ct:ct + 1],
                    )
                    ores = ostage.tile([128, width], F32, name="ores")
                    eng = nc.vector if t == N_TILES - 1 else nc.gpsimd
                    eng.tensor_tensor(
                        ores[:], obn[:], x_blk[ct][t][:, off:off + width], op=ADD
                    )
                    # last tile: keep everything on sync so the gpsimd DMA
                    # ring has nothing late to drain in the epilogue
                    q = nc.sync if (ct == 0 or t == N_TILES - 1) else nc.gpsimd
                    q.dma_start(
                        out[ct * 128:(ct + 1) * 128, t * NT + off:t * NT + off + width],
                        ores[:])

                def out_proj_mm_full(t, yt_tile):
                    o01 = st_ps.tile([128, 1024], F32, name="o01", tag="st")
                    for ct in range(2):
                        nc.tensor.matmul(
                            o01[:, ct * NT:(ct + 1) * NT],
                            wout_sb[:, ct * 128:(ct + 1) * 128],
                            yt_tile[:],
                            start=True, stop=True,
                            skip_group_check=True,
                        )
                    return [(o01[:, ct * NT:(ct + 1) * NT], ct, t, 0, NT)
                            for ct in range(2)]

                def out_proj_mm_half(t, h, yt_tile):
                    oh = st_ps.tile([128, 512], F32, name="oh", tag="st")
                    csl = slice(h * 256, (h + 1) * 256)
                    for ct in range(2):
                        nc.tensor.matmul(
                            oh[:, ct * 256:(ct + 1) * 256],
                            wout_sb[:, ct * 128:(ct + 1) * 128],
                            yt_tile[:, csl],
                            start=True, stop=True,
                            skip_group_check=True,
                        )
                    return [(oh[:, ct * 256:(ct + 1) * 256], ct, t, h * 256, 256)
                            for ct in range(2)]

                def y23_groups(pos):
                    # front-load the 11 groups of the previous tile's y23 mm2
                    # into positions 0..5 (their P^T tiles are all ready)
                    if pos < 5:
                        return [2 * pos, 2 * pos + 1]
                    if pos == 5:
                        return [10]
                    return []

                pts_prev = None
                yts = {}
                n_grps = len(GROUPS)
                last = N_TILES - 1
                for it in range(N_TILES + 1):
                    nt = it if it < N_TILES else None
                    pv = it - 1 if it >= 1 else None
                    if nt is not None:
                        ntsl = slice(nt * NT, (nt + 1) * NT)
                        y01 = ya.tile([128, 392], F32, name="y01", tag="ya")
                        yts[nt] = yt_pool.tile([O, NT], BF16, name=f"yt{nt}")
                    if pv is not None:
                        y23 = ya.tile([128, 392], F32, name="y23", tag="ya")
                    pts_cur = []
                    bns = []
                    for g in range(n_grps):
                        if pv is not None and g == 0:
                            # position 0: previous tile's y23 mm2 first (deps
                            # long met) so the PE has work while the previous
                            # out-projection's bn reads free the S^T-ring slot
                            for gg in y23_groups(0):
                                c0, c1 = GROUPS[gg]
                                ia = mm2(y23[:, 0:O + 1], pts_prev[gg], 2, c0, c1, c0)
                                ib = mm2(y23[:, 130:259], pts_prev[gg], 3, c0, c1, c0)
                                if c0 == 0:
                                    add_dep_helper(ib.ins, ia.ins, sync=False,
                                                   reason="bank-pack: clear before first write")
                        if nt is not None:
                            c0, c1 = GROUPS[g]
                            w = (c1 - c0) * NT
                            st = st_ps.tile([128, 3 * NT], F32, name="st", tag="st")
                            for c in range(c0, c1):
                                nc.tensor.matmul(
                                    st[:, (c - c0) * NT:(c - c0 + 1) * NT],
                                    phi_h[:, c * 128:(c + 1) * 128],
                                    theta_h[:, ntsl],
                                    start=True, stop=True,
                                )
                            pt = pt_pool.tile([128, 3 * NT], BF16)
                            if it == 0 and g < 4:
                                # pipeline fill: no pv work exists to hide a
                                # whole-group exp latency, so halve it by
                                # splitting the group across both engines
                                nc.scalar.activation(pt[:, :2 * NT], st[:, :2 * NT], EXP)
                                nc.vector.tensor_scalar(
                                    pt[:, 2 * NT:w].bitcast(I16), st[:, 2 * NT:w],
                                    FEXP_A, FEXP_B, op0=MULT, op1=ADD,
                                )
                            elif g in DVE_GROUPS:
                                nc.vector.tensor_scalar(
                                    pt[:, :w].bitcast(I16), st[:, :w],
                                    FEXP_A, FEXP_B, op0=MULT, op1=ADD,
                                )
                            else:
                                nc.scalar.activation(pt[:, :w], st[:, :w], EXP)
                            pts_cur.append(pt)
                            if g > 1:
                                # y01 mm2 lags TWO groups behind S^T: exp(g)
                                # then has ~2 pipeline positions before its
                                # consumer instead of 1 (a 3-chunk exp takes
                                # 1.4-1.7us but a back-half position is ~1us)
                                pc0, pc1 = GROUPS[g - 2]
                                ia = mm2(y01[:, 0:O + 1], pts_cur[g - 2], 0, pc0, pc1, pc0)
                                ib = mm2(y01[:, 130:259], pts_cur[g - 2], 1, pc0, pc1, pc0)
                                if pc0 == 0:
                                    add_dep_helper(ib.ins, ia.ins, sync=False,
                                                   reason="bank-pack: clear before first write")
                        if pv is not None and g > 0:
                            for gg in y23_groups(g):
                                c0, c1 = GROUPS[gg]
                                ia = mm2(y23[:, 0:O + 1], pts_prev[gg], 2, c0, c1, c0)
                                ib = mm2(y23[:, 130:259], pts_prev[gg], 3, c0, c1, c0)
                                if c0 == 0:
                                    add_dep_helper(ib.ins, ia.ins, sync=False,
                                                   reason="bank-pack: clear before first write")
                        if pv is not None and g == 6:
                            if pv == last:
                                # tail: first half of tile 7's output projection
                                # (yt cols 0:256 transposed during the previous
                                # iteration) straight after the final mm2 burst
                                bns += out_proj_mm_half(pv, 0, yts[pv])
                                norm_pair_pe(y23, yts[pv], 256, nc.scalar)
                            else:
                                norm_transpose(y23[:, 0:O + 1], yts[pv], 256, nc.sync)
                                norm_transpose(y23[:, 130:259], yts[pv], 384, nc.sync)
                        if pv is not None and g == 10:
                            if pv < last:
                                bns += out_proj_mm_full(pv, yts[pv])
                            else:
                                bns += out_proj_mm_half(pv, 1, yts[pv])
                    if nt is not None:
                        for gg in (n_grps - 2, n_grps - 1):
                            c0, c1 = GROUPS[gg]
                            mm2(y01[:, 0:O + 1], pts_cur[gg], 0, c0, c1, c0)
                            mm2(y01[:, 130:259], pts_cur[gg], 1, c0, c1, c0)
                    for args in bns:
                        bn_res_dma(*args)
                    if nt is not None:
                        if nt == last:
                            # copies on DVE: scalar is still draining exp/bn
                            # residue here, and these copies gate h0
                            norm_pair_pe(y01, yts[nt], 0, nc.vector)
                        else:
                            norm_transpose(y01[:, 0:O + 1], yts[nt], 0, nc.sync)
                            norm_transpose(y01[:, 130:259], yts[nt], 128, nc.sync)
                    pts_prev = pts_cur

    nc.finalize()
    return nc


_NC_CACHE = None


def _get_nc():
    global _NC_CACHE
    if _NC_CACHE is None:
        _NC_CACHE = build_nc()
    return _NC_CACHE


def _prepare_in_maps(inputs):
    x = np.ascontiguousarray(np.asarray(inputs["x"], dtype=np.float32)).reshape(B, C, N)
    xh = x.astype(np.float16)
    wth = np.ascontiguousarray(np.asarray(inputs["w_theta"], np.float32).T).astype(np.float16)
    wph = np.ascontiguousarray(np.asarray(inputs["w_phi"], np.float32).T).astype(np.float16)
    wg = np.ascontiguousarray(np.asarray(inputs["w_g"], np.float32).T).astype(np.float16)
    w_out = np.asarray(inputs["w_out"], np.float32)
    wout = np.ascontiguousarray(w_out.T).astype(ml_dtypes.bfloat16)
    bth = np.asarray(inputs["b_theta"], np.float32).reshape(O, 1)
    bph = np.asarray(inputs["b_phi"], np.float32).reshape(O, 1)
    inv = np.asarray(inputs["bn_gamma"], np.float32) / np.sqrt(
        np.asarray(inputs["bn_var"], np.float32) + BN_EPS)
    shift = (np.asarray(inputs["b_out"], np.float32) * inv
             + np.asarray(inputs["bn_beta"], np.float32)
             - np.asarray(inputs["bn_mean"], np.float32) * inv)
    # fold the g-branch bias through the output projection: softmax rows sum
    # to 1, so attn @ (g + b_g) = attn @ g + b_g, and w_out @ b_g is constant
    wob = wout.astype(np.float32) .T @ np.asarray(inputs["b_g"], np.float32)
    shift = shift + inv * wob
    bnscale = np.ascontiguousarray(inv.reshape(2, 128).T)
    bnshift = np.ascontiguousarray(shift.reshape(2, 128).T)

    shared = dict(wth=wth, wph=wph, wg=wg, wout=wout, bth=bth, bph=bph,
                  bnscale=bnscale, bnshift=bnshift,
                  identm=np.eye(128, dtype=ml_dtypes.bfloat16))
    return [dict(shared, xin=np.ascontiguousarray(xh[b])) for b in range(B)]


def _install_ntff_shim():
    """This image's antenv lacks axon_hooks; provide it from trn_boot's
    ctypes implementation so trace=True can capture NTFF profiles."""
    import types
    try:
        import antenv.axon_hooks  # noqa: F401
        return
    except ImportError:
        pass
    if "/root/.axon_site" not in sys.path:
        sys.path.insert(0, "/root/.axon_site")
    from trn_agent_boot.trn_boot import _ntff_profile_via_ctypes
    hook = _ntff_profile_via_ctypes("/opt/axon/libaxon_pjrt.so")
    m = types.ModuleType("antenv.axon_hooks")
    m.get_axon_ntff_profile_hook = lambda: hook
    m.set_axon_ntff_profile_hook = lambda h: None
    sys.modules["antenv.axon_hooks"] = m


def run(inputs, trace=False):
    if trace:
        _install_ntff_shim()
    nc = _get_nc()
    in_maps = _prepare_in_maps(inputs)
    res = run_bass_kernel_spmd(nc, in_maps, list(range(B)), trace=trace)
    outs = np.stack([res.results[b]["out"] for b in range(B)])
    return outs.reshape(B, C, 64, 64), res


def kernel(**inputs) -> np.ndarray:
    out, _ = run(inputs)
    return out


if __name__ == "__main__":
    # quick CoreSim check of one core
    from concourse import bass_interp
    rng = np.random.default_rng(0)
    fake = {
        "x": rng.standard_normal((B, C, 64, 64)).astype(np.float32),
        "w_theta": (rng.standard_normal((O, C)) * 0.05).astype(np.float32),
        "b_theta": (rng.standard_normal(O) * 0.05).astype(np.float32),
        "w_phi": (rng.standard_normal((O, C)) * 0.05).astype(np.float32),
        "b_phi": (rng.standard_normal(O) * 0.05).astype(np.float32),
        "w_g": (rng.standard_normal((O, C)) * 0.05).astype(np.float32),
        "b_g": (rng.standard_normal(O) * 0.05).astype(np.float32),
        "w_out": (rng.standard_normal((C, O)) * 0.05).astype(np.float32),
        "b_out": (rng.standard_normal(C) * 0.05).astype(np.float32),
        "bn_gamma": rng.standard_normal(C).astype(np.float32),
        "bn_beta": rng.standard_normal(C).astype(np.float32),
        "bn_mean": rng.standard_normal(C).astype(np.float32),
        "bn_var": rng.uniform(0.5, 1.5, C).astype(np.float32),
    }
    nc = _get_nc()
    in_maps = _prepare_in_maps(fake)
    sim = bass_interp.CoreSim(nc)
    for k, v in in_maps[0].items():
        sim.tensor(k)[:] = v
    sim.simulate()
    got = np.asarray(sim.tensor("out"))

    x0 = fake["x"][0].reshape(C, N)
    th = fake["w_theta"] @ x0 + fake["b_theta"][:, None]
    ph = fake["w_phi"] @ x0 + fake["b_phi"][:, None]
    gg = fake["w_g"] @ x0 + fake["b_g"][:, None]
    s = th.T @ ph
    p = np.exp(s - s.max(1, keepdims=True))
    a = p / p.sum(1, keepdims=True)
    yy = a @ gg.T
    wy = fake["w_out"] @ yy.T + fake["b_out"][:, None]
    inv = fake["bn_gamma"] / np.sqrt(fake["bn_var"] + BN_EPS)
    bn = wy * inv[:, None] + (fake["bn_beta"] - fake["bn_mean"] * inv)[:, None]
    want = x0 + bn
    err = np.abs(got - want).max()
    print("CoreSim absmax err:", err, "rel:", err / np.abs(want).max())
